# revision 1
# baseline (speedup 1.0000x reference)
"""MoE layer (top-2 of 8 experts) on 8 Trainium2 NeuronCores.

Strategy (expert-parallel, per the sharding hint):
  * Host computes the (tiny) gating network: probs = softmax(x @ w_gate),
    top-2 experts + normalized gates per token.  This is the sharding
    decision — it determines how tokens are dispatched to cores.
  * Tokens are dispatched by expert id: core e receives exactly the tokens
    routed to expert e (padded to a common capacity C), plus W1[e], W2[e]
    in bf16.  Each core runs the expert FFN  o = relu(x @ W1e) @ W2e  as a
    Bass/Tile kernel (bf16 matmuls, fp32 accumulation).
  * Host combines: y[n] = sum_k gate[n,k] * o_{e(n,k)}[slot(n,k)].

Device kernel layout (per core, SPMD over 8 cores):
  inputs  xT [D, C] bf16 (tokens transposed), w1 [D, F] bf16, w2 [F, D] bf16
  output  out [C, D] fp32
  mm1: hT[f,:]  = w1[:,f].T @ xT      (f on PSUM partitions, tokens moving)
  relu -> h_sb bf16
  mm2: out[t,:] += h_sb[:,t].T @ w2[fc,:]  (tokens on PSUM partitions,
       d moving, accumulated over all F in PSUM)
All weights stay resident in SBUF (16 MB bf16); tokens are processed in
blocks of TB=256 so the mm2 accumulators fit in 4 PSUM banks.
"""

import time

import numpy as np
import ml_dtypes

import concourse.bass as bass
import concourse.mybir as mybir
import concourse.tile as tile
from concourse import bacc
from concourse.bass_utils import run_bass_kernel_spmd

N, D, F, E, TOPK = 8192, 1024, 4096, 8, 2
P = 128
TB = 384          # tokens per block (3 PSUM m-tiles; 6 accum banks + 2 pipe)
NCORES = 8

BF16 = mybir.dt.bfloat16
F32 = mybir.dt.float32

_program_cache: dict[int, "bass.Bass"] = {}
LAST_RESULTS = None    # BassKernelResults of the most recent run (for test.py)
TRACE = False          # test.py can flip this before calling kernel()


def _build_program(C: int, bench_iters: int = 1) -> "bass.Bass":
    """One expert FFN: out[C, D] = relu(x @ W1) @ W2 with x given transposed.

    bench_iters > 1 wraps the compute in a hardware loop (same result, run
    repeatedly) so test harnesses can measure steady-state HW time from the
    wall-clock delta between two iteration counts."""
    KD = D // P            # 8  k-tiles over d_model
    KF = F // P            # 32 f-chunks of 128
    NB = C // TB           # token blocks
    TM = TB // P           # 2  PSUM m-tiles per block
    ND = D // 512          # 2  output n-tiles of 512
    QF = 4                 # weight chunks (pipelined load)
    KFQ = KF // QF         # 8 f-chunks per weight chunk

    nc = bacc.Bacc("TRN2", target_bir_lowering=False, debug=False,
                   num_devices=NCORES)
    xT = nc.dram_tensor("xT", [D, C], BF16, kind="ExternalInput")
    w1 = nc.dram_tensor("w1", [D, F], BF16, kind="ExternalInput")
    w2 = nc.dram_tensor("w2", [F, D], BF16, kind="ExternalInput")
    out = nc.dram_tensor("out", [C, D], F32, kind="ExternalOutput")

    xT_r = xT[:].rearrange("(ko p) n -> ko p n", p=P)
    w1_r = w1[:].rearrange("(ko p) f -> ko p f", p=P)
    w2_r = w2[:].rearrange("(ko p) d -> ko p d", p=P)
    out_r = out[:].rearrange("(nb tm p) d -> nb p tm d", tm=TM, p=P)

    with tile.TileContext(nc) as tc:
        with (
            tc.tile_pool(name="wpool", bufs=1) as wpool,
            tc.tile_pool(name="hpool", bufs=3) as hpool,
            tc.tile_pool(name="opool", bufs=2) as opool,
            tc.tile_pool(name="ph_pool", bufs=2, space="PSUM") as ph_pool,
            tc.tile_pool(name="po_pool", bufs=1, space="PSUM") as po_pool,
        ):
            xT_sb = wpool.tile([P, KD, C], BF16, name="xT_sb")
            for k in range(KD):
                nc.sync.dma_start(xT_sb[:, k, :], xT_r[k])

            # weights quartered along F so compute starts after 1/4 is loaded
            w1q = [wpool.tile([P, KD, KFQ * P], BF16, name=f"w1q{q}") for q in range(QF)]
            w2q = [wpool.tile([P, KFQ, D], BF16, name=f"w2q{q}") for q in range(QF)]
            for q in range(QF):
                for k in range(KD):
                    nc.sync.dma_start(
                        w1q[q][:, k, :], w1_r[k][:, q * KFQ * P:(q + 1) * KFQ * P]
                    )
                for k in range(KFQ):
                    nc.sync.dma_start(w2q[q][:, k, :], w2_r[q * KFQ + k])

            def token_block(t):
                po = [
                    [
                        po_pool.tile([P, 512], F32, name=f"po_{tm}_{nd}",
                                     tag=f"po_{tm}_{nd}")
                        for nd in range(ND)
                    ]
                    for tm in range(TM)
                ]

                def mm1(fc, t=t):
                    ph = ph_pool.tile([P, TB], F32, name="ph", tag="ph")
                    q, c = fc // KFQ, (fc % KFQ) * P
                    for ki in range(KD):
                        nc.tensor.matmul(
                            ph,
                            lhsT=w1q[q][:, ki, c:c + P],
                            rhs=xT_sb[:, ki, t * TB:(t + 1) * TB],
                            start=(ki == 0),
                            stop=(ki == KD - 1),
                        )
                    h = hpool.tile([P, TB], BF16, name="h", tag="h")
                    nc.scalar.activation(h, ph, mybir.ActivationFunctionType.Relu)
                    return h

                # software pipeline: emit mm1(fc+1) before mm2(fc) so the PE
                # never waits on the relu of the h-tile it is about to consume
                h_cur = mm1(0)
                for fc in range(KF):
                    h_next = mm1(fc + 1) if fc + 1 < KF else None
                    for tm in range(TM):
                        for nd in range(ND):
                            nc.tensor.matmul(
                                po[tm][nd],
                                lhsT=h_cur[:, tm * P:(tm + 1) * P],
                                rhs=w2q[fc // KFQ][:, fc % KFQ, nd * 512:(nd + 1) * 512],
                                start=(fc == 0),
                                stop=(fc == KF - 1),
                            )
                    h_cur = h_next

                o_sb = opool.tile([P, TM, D], F32, name="o_sb", tag="o_sb")
                for tm in range(TM):
                    for nd in range(ND):
                        nc.vector.tensor_copy(
                            o_sb[:, tm, nd * 512:(nd + 1) * 512], po[tm][nd]
                        )
                nc.sync.dma_start(out_r[t], o_sb[:])

            if bench_iters > 1:
                with tc.For_i(0, bench_iters, 1):
                    for t in range(NB):
                        token_block(t)
            else:
                for t in range(NB):
                    token_block(t)
    nc.compile()
    return nc


def _gate_and_dispatch(x, w_gate):
    """Replicates the reference gating exactly (fp32): softmax + top-2."""
    logits = x.astype(np.float32) @ w_gate.astype(np.float32)        # [N, E]
    m = logits.max(-1, keepdims=True)
    p = np.exp(logits - m)
    probs = p / p.sum(-1, keepdims=True)
    # jax.lax.top_k: descending, ties broken by lower index -> stable argsort
    tk_idx = np.argsort(-probs, axis=1, kind="stable")[:, :TOPK]
    tk_vals = np.take_along_axis(probs, tk_idx, axis=1)
    tk_gates = tk_vals / (tk_vals.sum(-1, keepdims=True) + 1e-9)
    return tk_idx, tk_gates


def kernel(x, w_gate, W1, W2):
    global LAST_RESULTS
    x = np.asarray(x, dtype=np.float32)
    w_gate = np.asarray(w_gate, dtype=np.float32)
    W1 = np.asarray(W1, dtype=np.float32)
    W2 = np.asarray(W2, dtype=np.float32)
    n_tok = x.shape[0]

    tk_idx, tk_gates = _gate_and_dispatch(x, w_gate)

    # dispatch: sort the n_tok*K (token, expert) assignments by expert id
    eid = tk_idx.reshape(-1).astype(np.int64)
    loads = np.bincount(eid, minlength=E)
    C = max(TB, int(np.ceil(loads.max() / TB)) * TB)

    order = np.argsort(eid, kind="stable")
    starts = np.zeros(E + 1, np.int64)
    starts[1:] = np.cumsum(loads)
    slot = np.empty(n_tok * TOPK, np.int64)
    slot[order] = np.arange(n_tok * TOPK) - starts[eid[order]]
    tok_of_flat = np.repeat(np.arange(n_tok), TOPK)

    # per-core inputs: tokens for expert e, transposed and padded to C
    in_maps = []
    for e in range(E):
        idx = tok_of_flat[order[starts[e]:starts[e + 1]]]
        xe_T = np.zeros((D, C), dtype=ml_dtypes.bfloat16)
        xe_T[:, :len(idx)] = np.ascontiguousarray(x[idx].T).astype(ml_dtypes.bfloat16)
        in_maps.append({
            "xT": xe_T,
            "w1": np.ascontiguousarray(W1[e]).astype(ml_dtypes.bfloat16),
            "w2": np.ascontiguousarray(W2[e]).astype(ml_dtypes.bfloat16),
        })

    nc = _program_cache.get(C)
    if nc is None:
        nc = _build_program(C)
        _program_cache[C] = nc

    try:
        res = run_bass_kernel_spmd(nc, in_maps, core_ids=list(range(NCORES)),
                                   trace=TRACE)
    except Exception:
        # transient NRT/device hiccups (e.g. NRT_EXEC_UNIT_UNRECOVERABLE)
        # have been observed to clear after a short wait — retry once
        time.sleep(20)
        res = run_bass_kernel_spmd(nc, in_maps, core_ids=list(range(NCORES)),
                                   trace=TRACE)
    LAST_RESULTS = res

    # combine: y[n] = sum_k gates[n,k] * out_{expert}[slot]
    O = np.concatenate([np.asarray(res.results[e]["out"]) for e in range(E)], axis=0)
    flat_rows = O[eid * C + slot]                       # [n_tok*K, D]
    y = (tk_gates.reshape(-1, 1) * flat_rows).reshape(n_tok, TOPK, D).sum(axis=1)
    return y.astype(np.float32)

